# revision 1
# baseline (speedup 1.0000x reference)
"""GQA attention (RoPE + ALiBi + causal) on 8 trn2 NeuronCores.

Sharding: core c -> batch b = c//4, kv-group g = c%4 (4 q-heads + 1 kv-head
per core, column-sharded Wq/Wk/Wv, row-sharded Wo; host sums the 4 partial
Wo outputs per batch).

All device matmuls run in float32r (full-rate PE) with N=512 moving dims.
Everything is kept transposed ([feature, token]) so softmax reductions over
keys become partition-dim reductions done with ones-vector matmuls, and the
per-key ALiBi column bias rides the exp() activation's per-partition bias.
The per-query ALiBi term is added with a K=1 ones matmul into the same PSUM
accumulation. Causal structure: only lower-triangle key tiles are computed;
diagonal tiles get the (transposed) mask block added before exp.
"""
import sys

if '/opt/trn_rl_repo' not in sys.path:
    sys.path.insert(0, '/opt/trn_rl_repo')

import numpy as np

B, T, D = 2, 2048, 2048
H, KV = 16, 4
HD = D // H          # 128
NREP = H // KV       # 4
KVD = 512            # per-core q width (4 heads x 128)
P = 128
TB = 512             # t-block
NBLK = T // TB       # 4
NC = D // P          # 16 contraction tiles
NJ = T // P          # 16 key tiles
ALIBI_W = 0.1
SCALE = (1.0 - ALIBI_W) / np.sqrt(np.float32(HD))

_cache = {}


def _build():
    from concourse import bacc, mybir
    from concourse.tile import TileContext

    F32 = mybir.dt.float32
    FR = mybir.dt.float32r
    EXP = mybir.ActivationFunctionType.Exp

    nc = bacc.Bacc()
    xT = nc.declare_dram_parameter("xT", [D, T], F32, isOutput=False)
    wq = nc.declare_dram_parameter("wq", [D, KVD], F32, isOutput=False)
    wk = nc.declare_dram_parameter("wk", [D, P], F32, isOutput=False)
    wv = nc.declare_dram_parameter("wv", [D, P], F32, isOutput=False)
    wo = nc.declare_dram_parameter("wo", [KVD, D], F32, isOutput=False)
    cosq = nc.declare_dram_parameter("cosq", [P, T], F32, isOutput=False)
    sinq = nc.declare_dram_parameter("sinq", [P, T], F32, isOutput=False)
    cosk = nc.declare_dram_parameter("cosk", [P, T], F32, isOutput=False)
    sink = nc.declare_dram_parameter("sink", [P, T], F32, isOutput=False)
    cb = nc.declare_dram_parameter("cb", [P, NREP * NBLK * NJ], F32, isOutput=False)
    maskT = nc.declare_dram_parameter("maskT", [P, 4 * TB], F32, isOutput=False)
    onesc = nc.declare_dram_parameter("onesc", [P, 1], F32, isOutput=False)
    idin = nc.declare_dram_parameter("idin", [P, P], F32, isOutput=False)
    out = nc.declare_dram_parameter("out", [T, D], F32, isOutput=True)

    with TileContext(nc) as tc:
        with (
            tc.tile_pool(name="const", bufs=1) as cpool,
            tc.tile_pool(name="kv", bufs=1) as kvpool,
            tc.tile_pool(name="tabs", bufs=1) as tpool,
            tc.tile_pool(name="xin", bufs=3) as xpool,
            tc.tile_pool(name="work", bufs=2) as wpool,
            tc.tile_pool(name="qt", bufs=4) as qpool,
            tc.tile_pool(name="pt", bufs=3) as ptpool,
            tc.tile_pool(name="ot", bufs=4) as opool,
            tc.tile_pool(name="ysb", bufs=2) as ypool,
            tc.tile_pool(name="small", bufs=2) as spool,
            tc.tile_pool(name="ps", bufs=1, space="PSUM") as pss,
        ):
            # ---- resident constants ----
            wq_sb = cpool.tile([P, NC, KVD], FR)
            wq_r = wq.rearrange("(c p) n -> p c n", p=P).bitcast(FR)
            for c in range(NC):
                nc.sync.dma_start(out=wq_sb[:, c], in_=wq_r[:, c])
            wk_sb = cpool.tile([P, NC, P], FR)
            wk_r = wk.rearrange("(c p) n -> p c n", p=P).bitcast(FR)
            wv_sb = cpool.tile([P, NC, P], FR)
            wv_r = wv.rearrange("(c p) n -> p c n", p=P).bitcast(FR)
            for c4 in range(4):
                nc.sync.dma_start(out=wk_sb[:, c4 * 4:(c4 + 1) * 4], in_=wk_r[:, c4 * 4:(c4 + 1) * 4])
                nc.sync.dma_start(out=wv_sb[:, c4 * 4:(c4 + 1) * 4], in_=wv_r[:, c4 * 4:(c4 + 1) * 4])
            wo_sb = cpool.tile([P, NREP, D], FR)
            wo_r = wo.rearrange("(h p) e -> p h e", p=P).bitcast(FR)
            for h in range(NREP):
                nc.sync.dma_start(out=wo_sb[:, h], in_=wo_r[:, h])
            cb_sb = cpool.tile([P, NREP * NBLK * NJ], F32)
            nc.sync.dma_start(out=cb_sb, in_=cb[:, :])
            maskT_sb = cpool.tile([P, 4 * TB], F32)
            nc.sync.dma_start(out=maskT_sb, in_=maskT[:, :])
            onesc_sb = cpool.tile([P, 1], FR)
            nc.sync.dma_start(out=onesc_sb, in_=onesc[:, :].bitcast(FR))
            id_sb = cpool.tile([P, P], FR)
            nc.sync.dma_start(out=id_sb, in_=idin[:, :].bitcast(FR))

            kT_sb = kvpool.tile([P, T], FR)          # roped K, [d, s]
            v_sb = kvpool.tile([P, NJ, P], FR)       # V tiles, [s, j, d']

            for bk in range(NBLK):
                t0 = bk * TB
                # ---- tables for this block ----
                cq_t = tpool.tile([P, TB], F32, tag="cq")
                nc.sync.dma_start(out=cq_t, in_=cosq[:, t0:t0 + TB])
                sq_t = tpool.tile([P, TB], F32, tag="sq")
                nc.sync.dma_start(out=sq_t, in_=sinq[:, t0:t0 + TB])
                ck_t = tpool.tile([P, TB], F32, tag="ck")
                nc.sync.dma_start(out=ck_t, in_=cosk[:, t0:t0 + TB])
                sk_t = tpool.tile([P, TB], F32, tag="sk")
                nc.sync.dma_start(out=sk_t, in_=sink[:, t0:t0 + TB])

                # ---- projections ----
                q_ps = [None] * NREP
                q_ps[0] = pss.tile([P, TB], F32, tag="big", bufs=7, name=f"qps{bk}_0")
                q_ps[1] = pss.tile([P, TB], F32, tag="big", bufs=7, name=f"qps{bk}_1")
                k_ps = pss.tile([P, TB], F32, tag="big", bufs=7, name=f"kps{bk}")
                v_ps = pss.tile([P, TB], F32, tag="big", bufs=7, name=f"vps{bk}")
                for c in range(NC):
                    xt = xpool.tile([P, TB], FR, tag="xt", name=f"xtA{bk}_{c}")
                    nc.sync.dma_start(out=xt, in_=xT[c * P:(c + 1) * P, t0:t0 + TB].bitcast(FR))
                    for h in (0, 1):
                        nc.tensor.matmul(q_ps[h], wq_sb[:, c, h * P:(h + 1) * P], xt,
                                         start=(c == 0), stop=(c == NC - 1))
                    nc.tensor.matmul(k_ps, wk_sb[:, c, :], xt, start=(c == 0), stop=(c == NC - 1))
                    nc.tensor.matmul(v_ps, wv_sb[:, c, :], xt, start=(c == 0), stop=(c == NC - 1))
                q_ps[2] = pss.tile([P, TB], F32, tag="big", bufs=7, name=f"qps{bk}_2")
                q_ps[3] = pss.tile([P, TB], F32, tag="big", bufs=7, name=f"qps{bk}_3")
                for c in range(NC):
                    xt = xpool.tile([P, TB], FR, tag="xt", name=f"xtB{bk}_{c}")
                    nc.sync.dma_start(out=xt, in_=xT[c * P:(c + 1) * P, t0:t0 + TB].bitcast(FR))
                    for h in (2, 3):
                        nc.tensor.matmul(q_ps[h], wq_sb[:, c, h * P:(h + 1) * P], xt,
                                         start=(c == 0), stop=(c == NC - 1))

                # ---- RoPE ----
                def rope(dst, src_ps, cos_t, sin_t, nm):
                    raw = wpool.tile([P, TB], F32, tag="raw", name=f"raw{nm}")
                    nc.scalar.copy(raw, src_ps)
                    swp = wpool.tile([P, TB], F32, tag="swp", name=f"swp{nm}")
                    nc.sync.dma_start(out=swp[0:64, :], in_=raw[64:128, :])
                    nc.sync.dma_start(out=swp[64:128, :], in_=raw[0:64, :])
                    m1 = wpool.tile([P, TB], F32, tag="m1", name=f"m1{nm}")
                    nc.vector.tensor_mul(m1, src_ps, cos_t)
                    m2 = wpool.tile([P, TB], F32, tag="m2", name=f"m2{nm}")
                    nc.vector.tensor_mul(m2, swp, sin_t)
                    nc.vector.tensor_add(dst, m1, m2)

                q_sb = []
                for h in range(NREP):
                    qh = qpool.tile([P, TB], FR, tag="qT", name=f"qT{bk}_{h}")
                    rope(qh, q_ps[h], cq_t, sq_t, f"q{bk}_{h}")
                    q_sb.append(qh)
                rope(kT_sb[:, t0:t0 + TB], k_ps, ck_t, sk_t, f"k{bk}")

                # ---- V: copy + transpose to [s, d'] ----
                vtmp = wpool.tile([P, TB], FR, tag="vtmp", name=f"vtmp{bk}")
                nc.scalar.copy(vtmp, v_ps)
                for sj in range(4):
                    vt_ps = pss.tile([P, P], FR, tag="big", bufs=7, name=f"vtps{bk}_{sj}")
                    nc.tensor.transpose(vt_ps, vtmp[:, sj * P:(sj + 1) * P], id_sb)
                    nc.vector.tensor_copy(v_sb[:, 4 * bk + sj, :], vt_ps)

                # ---- attention ----
                nj = 4 * bk + 4
                for h in range(NREP):
                    ot_ps = pss.tile([P, TB], F32, tag="big", bufs=7, name=f"otps{bk}_{h}")
                    cs_ps = pss.tile([1, TB], F32, tag="cs", bufs=1, name=f"csps{bk}_{h}")
                    for j in range(nj):
                        s_ps = pss.tile([P, TB], F32, tag="big", bufs=7, name=f"sps{bk}_{h}_{j}")
                        nc.tensor.matmul(s_ps, kT_sb[:, j * P:(j + 1) * P], q_sb[h],
                                         start=True, stop=True)
                        delta = j - 4 * bk
                        if delta >= 0:
                            nc.vector.tensor_add(s_ps, s_ps,
                                                 maskT_sb[:, delta * TB:(delta + 1) * TB])
                        pt = ptpool.tile([P, TB], FR, tag="pt", name=f"pt{bk}_{h}_{j}")
                        nc.scalar.activation(pt, s_ps, EXP,
                                             bias=cb_sb[:, (h * NBLK + bk) * NJ + j:(h * NBLK + bk) * NJ + j + 1])
                        nc.tensor.matmul(cs_ps, onesc_sb, pt,
                                         start=(j == 0), stop=(j == nj - 1))
                        nc.tensor.matmul(ot_ps, v_sb[:, j, :], pt,
                                         start=(j == 0), stop=(j == nj - 1))
                    rec = spool.tile([1, TB], F32, tag="rec", name=f"rec{bk}_{h}")
                    nc.vector.reciprocal(rec, cs_ps)
                    rbc = spool.tile([P, TB], F32, tag="rbc", name=f"rbc{bk}_{h}")
                    nc.gpsimd.partition_broadcast(rbc, rec)
                    oh = opool.tile([P, TB], FR, tag="ot", name=f"ot{bk}_{h}")
                    nc.vector.tensor_mul(oh, ot_ps, rbc)
                    q_sb[h] = oh  # reuse list slot to keep handles

                ot_sb = q_sb  # [h] -> [d', t] normalized attention out

                # ---- Wo partial ----
                for ts_ in range(4):
                    for e in range(4):
                        y_ps = pss.tile([P, TB], F32, tag="big", bufs=7, name=f"yps{bk}_{ts_}_{e}")
                        for h in range(NREP):
                            nc.tensor.matmul(y_ps, ot_sb[h][:, ts_ * P:(ts_ + 1) * P],
                                             wo_sb[:, h, e * TB:(e + 1) * TB],
                                             start=(h == 0), stop=(h == NREP - 1))
                        y_sb = ypool.tile([P, TB], F32, tag="ysb", name=f"y{bk}_{ts_}_{e}")
                        nc.vector.tensor_copy(y_sb, y_ps)
                        nc.sync.dma_start(
                            out=out[t0 + ts_ * P:t0 + (ts_ + 1) * P, e * TB:(e + 1) * TB],
                            in_=y_sb)

    nc.compile()
    return nc


def _prep_inputs(x, mask, freqs_cis, alibi_bias, Wq, Wk, Wv, Wo):
    """Host-side prep: transposes, RoPE tables, ALiBi bias decomposition."""
    f64 = np.float64
    idx = np.arange(HD)
    cos_full = freqs_cis[:, idx // 2]                     # [T, 128]
    sin_full = freqs_cis[:, (HD // 2) + idx // 2]         # [T, 128]
    sign = np.where(idx < HD // 2, -1.0, 1.0).astype(np.float32)
    cosT = np.ascontiguousarray(cos_full.T)               # [128, T]
    sinT_signed = np.ascontiguousarray((sin_full * sign[None, :]).T)

    cosq = (cosT * np.float32(SCALE)).astype(np.float32)
    sinq = (sinT_signed * np.float32(SCALE)).astype(np.float32)
    cosk = cosT.astype(np.float32)
    sink = sinT_signed.astype(np.float32)

    m = mask[0, 0]
    maskT = np.empty((P, 4 * TB), np.float32)
    for d in range(4):
        maskT[:, d * TB:(d + 1) * TB] = m[:TB, d * P:(d + 1) * P].T

    onesc = np.ones((P, 1), np.float32)
    idin = np.eye(P, dtype=np.float32)

    in_maps = []
    for c in range(8):
        b, g = c // 4, c % 4
        slopes = np.array([-f64(alibi_bias[0, g * NREP + hl, 1, 0]) for hl in range(NREP)])
        pvec = np.arange(P, dtype=f64)
        jvec = np.arange(NJ, dtype=f64)
        # cb[p, h, bk, j] = ALIBI_W*slope*(j*128 + p) - ALIBI_W*slope*(bk*512 + 511)
        bkvec = np.arange(NBLK, dtype=f64)
        cbv = (ALIBI_W * slopes[:, None, None, None]
               * (jvec[None, None, :, None] * P + pvec[None, None, None, :]
                  - (bkvec[None, :, None, None] * TB + (TB - 1))))
        cbm = np.ascontiguousarray(cbv.transpose(3, 0, 1, 2).reshape(P, NREP * NBLK * NJ)).astype(np.float32)
        in_maps.append({
            "xT": np.ascontiguousarray(x[b].T),
            "wq": np.ascontiguousarray(Wq[:, g * KVD:(g + 1) * KVD]),
            "wk": np.ascontiguousarray(Wk[:, g * P:(g + 1) * P]),
            "wv": np.ascontiguousarray(Wv[:, g * P:(g + 1) * P]),
            "wo": np.ascontiguousarray(Wo[g * KVD:(g + 1) * KVD, :]),
            "cosq": cosq, "sinq": sinq, "cosk": cosk, "sink": sink,
            "cb": cbm, "maskT": maskT,
            "onesc": onesc, "idin": idin,
        })
    return in_maps


def kernel(x, mask, freqs_cis, alibi_bias, Wq, Wk, Wv, Wo, _trace=False, _trace_kwargs=None):
    from concourse.bass_utils import run_bass_kernel_spmd

    if "nc" not in _cache:
        _cache["nc"] = _build()
    nc = _cache["nc"]

    in_maps = _prep_inputs(np.asarray(x, np.float32), np.asarray(mask, np.float32),
                           np.asarray(freqs_cis, np.float32), np.asarray(alibi_bias, np.float32),
                           np.asarray(Wq, np.float32), np.asarray(Wk, np.float32),
                           np.asarray(Wv, np.float32), np.asarray(Wo, np.float32))
    kw = {}
    if _trace:
        kw = dict(trace=True, **(_trace_kwargs or {}))
    res = run_bass_kernel_spmd(nc, in_maps, list(range(8)), **kw)

    full = np.zeros((B, T, D), np.float32)
    for c in range(8):
        full[c // 4] += res.results[c]["out"]
    if _trace:
        _cache["last_trace"] = res
    return full



# revision 5
# speedup vs baseline: 1.7317x; 1.7317x over previous
"""GQA attention (RoPE + ALiBi + causal) on 8 trn2 NeuronCores.

Sharding: core c -> batch b = c//4, kv-group g = c%4 (4 q-heads + 1 kv-head
per core, column-sharded Wq/Wk/Wv, row-sharded Wo; host sums the 4 partial
Wo outputs per batch).

v2: software-pipelined phase emission (proj A / attention B / out-proj C
interleaved A0,A1,B0,A2,B1,C0,A3,B2,C1,B3,C2,C3) so the PE instruction
queue always has runnable matmuls and the HAM clock gate stays warm.
All matmuls run in bf16 (same 1 cyc/row PE rate as fp32r, half the DMA/
SBUF/LDWEIGHTS cost). V is produced directly transposed by using the x
tiles as the stationary operand. Softmax scale rides the exp activation's
`scale` (mask pre-divided by it on host); per-key ALiBi rides the exp
bias; the per-query ALiBi residual cancels in softmax. Normalization uses
reciprocal_approx_fast. Score/exp/reduce emission is staggered so the PE
never waits on the scalar engine's exp.
"""
import sys

if '/opt/trn_rl_repo' not in sys.path:
    sys.path.insert(0, '/opt/trn_rl_repo')

import numpy as np
import ml_dtypes

BF = ml_dtypes.bfloat16

B, T, D = 2, 2048, 2048
H, KV = 16, 4
HD = D // H          # 128
NREP = H // KV       # 4
KVD = 512            # per-core q width (4 heads x 128)
P = 128
TB = 512             # t-block
NBLK = T // TB       # 4
NC = D // P          # 16 contraction tiles
NJ = T // P          # 16 key tiles
ALIBI_W = 0.1
SCALE = float((1.0 - ALIBI_W) / np.sqrt(np.float32(HD)))

_cache = {}


def _build():
    from concourse import bacc, mybir
    from concourse.tile import TileContext

    F32 = mybir.dt.float32
    BF16 = mybir.dt.bfloat16
    EXP = mybir.ActivationFunctionType.Exp

    nc = bacc.Bacc()
    xT = nc.declare_dram_parameter("xT", [D, T], BF16, isOutput=False)
    wq = nc.declare_dram_parameter("wq", [D, KVD], BF16, isOutput=False)
    wk = nc.declare_dram_parameter("wk", [D, P], BF16, isOutput=False)
    wv = nc.declare_dram_parameter("wv", [D, P], BF16, isOutput=False)
    wo = nc.declare_dram_parameter("wo", [KVD, D], BF16, isOutput=False)
    cosT = nc.declare_dram_parameter("cosT", [P, T], F32, isOutput=False)
    sinT = nc.declare_dram_parameter("sinT", [P, T], F32, isOutput=False)
    cb = nc.declare_dram_parameter("cb", [P, NREP * NBLK * NJ], F32, isOutput=False)
    maskT = nc.declare_dram_parameter("maskT", [P, 4 * TB], F32, isOutput=False)
    onesc = nc.declare_dram_parameter("onesc", [P, 1], BF16, isOutput=False)
    out = nc.declare_dram_parameter("out", [T, D], BF16, isOutput=True)

    with TileContext(nc) as tc:
        with (
            tc.tile_pool(name="const", bufs=1) as cpool,
            tc.tile_pool(name="kv", bufs=1) as kvpool,
            tc.tile_pool(name="xin", bufs=24) as xpool,
            tc.tile_pool(name="rope", bufs=3) as rpool,
            tc.tile_pool(name="qt", bufs=12) as qpool,
            tc.tile_pool(name="pt", bufs=4) as ptpool,
            tc.tile_pool(name="oh", bufs=8) as opool,
            tc.tile_pool(name="ysb", bufs=4) as ypool,
            tc.tile_pool(name="small", bufs=2) as spool,
            tc.tile_pool(name="ps", bufs=1, space="PSUM") as pss,
        ):
            # ---- resident constants ----
            wq_sb = cpool.tile([P, NC, KVD], BF16)
            wq_r = wq.rearrange("(c p) n -> p c n", p=P)
            for c in range(NC):
                nc.sync.dma_start(out=wq_sb[:, c], in_=wq_r[:, c])
            wk_sb = cpool.tile([P, NC, P], BF16)
            wk_r = wk.rearrange("(c p) n -> p c n", p=P)
            wv_sb = cpool.tile([P, NC, P], BF16)
            wv_r = wv.rearrange("(c p) n -> p c n", p=P)
            for c4 in range(4):
                nc.sync.dma_start(out=wk_sb[:, c4 * 4:(c4 + 1) * 4], in_=wk_r[:, c4 * 4:(c4 + 1) * 4])
                nc.sync.dma_start(out=wv_sb[:, c4 * 4:(c4 + 1) * 4], in_=wv_r[:, c4 * 4:(c4 + 1) * 4])
            wo_sb = cpool.tile([P, NREP, D], BF16)
            wo_r = wo.rearrange("(h p) e -> p h e", p=P)
            for h in range(NREP):
                nc.sync.dma_start(out=wo_sb[:, h], in_=wo_r[:, h])
            cos_sb = cpool.tile([P, T], F32)
            nc.sync.dma_start(out=cos_sb, in_=cosT[:, :])
            sin_sb = cpool.tile([P, T], F32)
            nc.sync.dma_start(out=sin_sb, in_=sinT[:, :])
            cb_sb = cpool.tile([P, NREP * NBLK * NJ], F32)
            nc.sync.dma_start(out=cb_sb, in_=cb[:, :])
            maskT_sb = cpool.tile([P, 4 * TB], F32)
            nc.sync.dma_start(out=maskT_sb, in_=maskT[:, :])
            onesc_sb = cpool.tile([P, 1], BF16)
            nc.sync.dma_start(out=onesc_sb, in_=onesc[:, :])

            kT_sb = kvpool.tile([P, T], BF16)        # roped K, [d, s]
            v_sb = kvpool.tile([P, NJ * P], BF16)    # V transposed, [s_local, j*128+d']

            q_sb = {}    # (bk, h) -> [d', t] bf16 roped q
            oh_sb = {}   # (bk, h) -> [d', t] bf16 normalized attention out

            def rope(dst, src_ps, t0, nm):
                raw = rpool.tile([P, TB], F32, tag="raw", name=f"raw{nm}")
                nc.scalar.copy(raw, src_ps)
                swp = rpool.tile([P, TB], F32, tag="swp", name=f"swp{nm}")
                nc.sync.dma_start(out=swp[0:64, :], in_=raw[64:128, :])
                nc.sync.dma_start(out=swp[64:128, :], in_=raw[0:64, :])
                m1 = rpool.tile([P, TB], F32, tag="m1", name=f"m1{nm}")
                nc.vector.tensor_mul(m1, src_ps, cos_sb[:, t0:t0 + TB])
                m2 = rpool.tile([P, TB], F32, tag="m2", name=f"m2{nm}")
                nc.vector.tensor_mul(m2, swp, sin_sb[:, t0:t0 + TB])
                nc.vector.tensor_add(dst, m1, m2)

            def phase_a(bk):
                """projections + rope for t-block bk"""
                t0 = bk * TB
                q_ps = [pss.tile([P, TB], F32, tag="big", bufs=6, name=f"qps{bk}_{h}")
                        for h in range(NREP)]
                k_ps = pss.tile([P, TB], F32, tag="big", bufs=6, name=f"kps{bk}")
                xts = []
                for c in range(NC):
                    xt = xpool.tile([P, TB], BF16, tag="xt", name=f"xt{bk}_{c}")
                    nc.sync.dma_start(out=xt, in_=xT[c * P:(c + 1) * P, t0:t0 + TB])
                    xts.append(xt)
                    for h in range(NREP):
                        nc.tensor.matmul(q_ps[h], wq_sb[:, c, h * P:(h + 1) * P], xt,
                                         start=(c == 0), stop=(c == NC - 1))
                    nc.tensor.matmul(k_ps, wk_sb[:, c, :], xt,
                                     start=(c == 0), stop=(c == NC - 1))
                # V transposed: ts-outer so each bank quarter finishes its full
                # c-accumulation before the next quarter's start=True clears
                # the bank's has_written bits (data in other columns persists)
                vt_ps = pss.tile([P, TB], F32, tag="vt", bufs=1, name=f"vtps{bk}")
                for ts_ in range(4):
                    for c in range(NC):
                        nc.tensor.matmul(vt_ps[:, ts_ * P:(ts_ + 1) * P],
                                         xts[c][:, ts_ * P:(ts_ + 1) * P], wv_sb[:, c, :],
                                         start=(c == 0), stop=(c == NC - 1))
                nc.vector.tensor_copy(v_sb[:, bk * TB:(bk + 1) * TB], vt_ps)
                for h in range(NREP):
                    qh = qpool.tile([P, TB], BF16, tag="qT", name=f"qT{bk}_{h}")
                    rope(qh, q_ps[h], t0, f"q{bk}_{h}")
                    q_sb[(bk, h)] = qh
                rope(kT_sb[:, t0:t0 + TB], k_ps, t0, f"k{bk}")

            def phase_b(bk):
                """attention for q-block bk over key tiles 0..4*bk+3"""
                nj = 4 * bk + 4
                for h in range(NREP):
                    ot_ps = pss.tile([P, TB], F32, tag="big", bufs=6, name=f"otps{bk}_{h}")
                    cs_ps = pss.tile([1, TB], F32, tag="cs", bufs=1, name=f"csps{bk}_{h}")
                    q = q_sb[(bk, h)]
                    pts = {}

                    def emit_s(j):
                        s_ps = pss.tile([P, TB], F32, tag="big", bufs=6,
                                        name=f"sps{bk}_{h}_{j}")
                        nc.tensor.matmul(s_ps, kT_sb[:, j * P:(j + 1) * P], q,
                                         start=True, stop=True)
                        delta = j - 4 * bk
                        if delta >= 0:
                            nc.vector.tensor_add(s_ps, s_ps,
                                                 maskT_sb[:, delta * TB:(delta + 1) * TB])
                        pt = ptpool.tile([P, TB], BF16, tag="pt", name=f"pt{bk}_{h}_{j}")
                        col = (h * NBLK + bk) * NJ + j
                        nc.scalar.activation(pt, s_ps, EXP,
                                             bias=cb_sb[:, col:col + 1], scale=SCALE)
                        pts[j] = pt

                    def emit_red(j):
                        nc.tensor.matmul(cs_ps, onesc_sb, pts[j],
                                         start=(j == 0), stop=(j == nj - 1))
                        nc.tensor.matmul(ot_ps, v_sb[:, j * P:(j + 1) * P], pts[j],
                                         start=(j == 0), stop=(j == nj - 1))

                    emit_s(0)
                    emit_s(1)
                    for j in range(nj):
                        if j + 2 < nj:
                            emit_s(j + 2)
                        emit_red(j)

                    rec = spool.tile([1, TB], F32, tag="rec", name=f"rec{bk}_{h}")
                    nc.vector.reciprocal_approx_fast(rec, cs_ps)
                    rbc = spool.tile([P, TB], F32, tag="rbc", name=f"rbc{bk}_{h}")
                    nc.gpsimd.partition_broadcast(rbc, rec)
                    oh = opool.tile([P, TB], BF16, tag="oh", name=f"oh{bk}_{h}")
                    nc.vector.tensor_mul(oh, ot_ps, rbc)
                    oh_sb[(bk, h)] = oh

            def phase_c(bk):
                """output projection partial for t-block bk"""
                t0 = bk * TB
                for ts_ in range(4):
                    for e in range(4):
                        y_ps = pss.tile([P, TB], F32, tag="big", bufs=6,
                                        name=f"yps{bk}_{ts_}_{e}")
                        for h in range(NREP):
                            nc.tensor.matmul(y_ps,
                                             oh_sb[(bk, h)][:, ts_ * P:(ts_ + 1) * P],
                                             wo_sb[:, h, e * TB:(e + 1) * TB],
                                             start=(h == 0), stop=(h == NREP - 1))
                        y_sb = ypool.tile([P, TB], BF16, tag="ysb", name=f"y{bk}_{ts_}_{e}")
                        nc.vector.tensor_copy(y_sb, y_ps)
                        nc.sync.dma_start(
                            out=out[t0 + ts_ * P:t0 + (ts_ + 1) * P, e * TB:(e + 1) * TB],
                            in_=y_sb)

            # software-pipelined emission: keep the PE queue dense across phases
            phase_a(0)
            phase_a(1)
            phase_b(0)
            phase_a(2)
            phase_b(1)
            phase_c(0)
            phase_a(3)
            phase_b(2)
            phase_c(1)
            phase_b(3)
            phase_c(2)
            phase_c(3)

    nc.compile()
    return nc


def _prep_inputs(x, mask, freqs_cis, alibi_bias, Wq, Wk, Wv, Wo):
    """Host-side prep: transposes, RoPE tables, ALiBi bias decomposition."""
    f64 = np.float64
    idx = np.arange(HD)
    cos_full = freqs_cis[:, idx // 2]                     # [T, 128]
    sin_full = freqs_cis[:, (HD // 2) + idx // 2]         # [T, 128]
    sign = np.where(idx < HD // 2, -1.0, 1.0).astype(np.float32)
    cosT = np.ascontiguousarray(cos_full.T).astype(np.float32)        # [128, T]
    sinT = np.ascontiguousarray((sin_full * sign[None, :]).T).astype(np.float32)

    m = mask[0, 0]
    maskT = np.empty((P, 4 * TB), np.float32)
    for d in range(4):
        maskT[:, d * TB:(d + 1) * TB] = m[:TB, d * P:(d + 1) * P].T
    maskT *= np.float32(1.0 / SCALE)    # scale rides the exp activation

    onesc = np.ones((P, 1), BF)

    in_maps = []
    for c in range(8):
        b, g = c // 4, c % 4
        slopes = np.array([-f64(alibi_bias[0, g * NREP + hl, 1, 0]) for hl in range(NREP)])
        pvec = np.arange(P, dtype=f64)
        jvec = np.arange(NJ, dtype=f64)
        # cb[p, h, bk, j] = ALIBI_W*slope*(j*128 + p) - ALIBI_W*slope*(bk*512 + 511)
        bkvec = np.arange(NBLK, dtype=f64)
        cbv = (ALIBI_W * slopes[:, None, None, None]
               * (jvec[None, None, :, None] * P + pvec[None, None, None, :]
                  - (bkvec[None, :, None, None] * TB + (TB - 1))))
        cbm = np.ascontiguousarray(cbv.transpose(3, 0, 1, 2).reshape(P, NREP * NBLK * NJ)).astype(np.float32)
        in_maps.append({
            "xT": np.ascontiguousarray(x[b].T).astype(BF),
            "wq": np.ascontiguousarray(Wq[:, g * KVD:(g + 1) * KVD]).astype(BF),
            "wk": np.ascontiguousarray(Wk[:, g * P:(g + 1) * P]).astype(BF),
            "wv": np.ascontiguousarray(Wv[:, g * P:(g + 1) * P]).astype(BF),
            "wo": np.ascontiguousarray(Wo[g * KVD:(g + 1) * KVD, :]).astype(BF),
            "cosT": cosT, "sinT": sinT,
            "cb": cbm, "maskT": maskT,
            "onesc": onesc,
        })
    return in_maps


def kernel(x, mask, freqs_cis, alibi_bias, Wq, Wk, Wv, Wo, _trace=False, _trace_kwargs=None):
    from concourse.bass_utils import run_bass_kernel_spmd

    if "nc" not in _cache:
        _cache["nc"] = _build()
    nc = _cache["nc"]

    in_maps = _prep_inputs(np.asarray(x, np.float32), np.asarray(mask, np.float32),
                           np.asarray(freqs_cis, np.float32), np.asarray(alibi_bias, np.float32),
                           np.asarray(Wq, np.float32), np.asarray(Wk, np.float32),
                           np.asarray(Wv, np.float32), np.asarray(Wo, np.float32))
    kw = {}
    if _trace:
        kw = dict(trace=True, **(_trace_kwargs or {}))
    res = run_bass_kernel_spmd(nc, in_maps, list(range(8)), **kw)

    full = np.zeros((B, T, D), np.float32)
    for c in range(8):
        full[c // 4] += res.results[c]["out"].astype(np.float32)
    if _trace:
        _cache["last_trace"] = res
    return full
